# revision 32
# baseline (speedup 1.0000x reference)
"""Multi-head attention block (nn_Attention) on 8 Trainium2 NeuronCores.

Reference computation (per batch element, all fp32):
    qkv = x @ w_qkv.T + b_qkv               # [T, 3D]
    q, k, v per head (H=12, Hd=64)
    attn = softmax(q @ k.T / sqrt(Hd))
    out  = (attn @ v) @ w_proj.T + b_proj   # [T, D]

Sharding: pure data parallelism over the batch (B=8) — one batch element per
NeuronCore, weights replicated. No collectives.

All matmuls run in float32r (fp32 storage, TF32-like PE mode: full rate for
moving dim >= 256, ~1.6e-4 matmul relative error). x and the weights are
pre-transposed on the host so every operand DMAs in with the contraction
dim on partitions and unit-stride free dims:
    xT  [D, T],  wT_qkv [D, 3D],  wT_proj [D, D]

Per-core pipeline:
  1. qkT [1536, T] = wT_qk.T-contract against xT (features on partitions),
     bias folded into the PSUM evacuation. v_nat [T, 768] = x @ w_v.T,
     staged head-major as [v_h | 1] blocks of 65 columns (the ones column
     makes the attention matmul emit softmax denominators for free).
  2. Per head pair (heads 2i, 2i+1 live at partition bases 0/64 of one
     qkT tile, so their K=64 S.T matmuls occupy distinct PE row groups and
     run concurrently): S.T = kT_h.T @ qT_h per 128-key tile, exp on
     ScalarE (scale=1/8 folded, output rounded to f32r), then
     O'.T [65, tq] = [v_h | 1].T @ P.T accumulated over the 8 key tiles.
     Row 64 of O'.T is the softmax denominator. The evacuation multiplies
     rows 0:63 by the broadcast reciprocal (GPSIMD partition_broadcast)
     into OT [D, T].
  3. out = OT.T-contract against wT_proj + b_proj, written token-major.

QKV matmuls for head pair i+1 are emitted between attention stages of pair
i so the PE stays busy while ScalarE grinds the exps (ScalarE is the
attention-phase bottleneck at ~1 elem/lane/cycle).
"""
import os
import numpy as np

os.environ.setdefault("JAX_COMPILATION_CACHE_DIR", "/tmp/jax_neff_cache")

import concourse.bass as bass
import concourse.bacc as bacc
import concourse.tile as tile
from concourse import mybir

F32 = mybir.dt.float32
F32R = mybir.dt.float32r

B, T, D = 8, 1024, 768
H, HD = 12, 64
SCALE = HD ** -0.5
N_CORES = 8
TT = T // 128       # 8 token tiles
DT = D // 128       # 6 contraction tiles
TQ = 512            # query chunk (moving dim)
NCH = T // TQ       # 2 query chunks
SG = [(0, 2), (2, 4), (4, 6), (6, 8)]  # key-tile groups (2 PSUM banks each)


def _bcast_ap(ap_1d, parts, n):
    return bass.AP(tensor=ap_1d.tensor, offset=ap_1d.offset,
                   ap=[[0, parts], [1, n]])


def build_nc():
    nc = bacc.Bacc(trn_type="TRN2", debug=False, num_devices=N_CORES)
    xt_d = nc.dram_tensor("xT", (D, T), F32, kind="ExternalInput")
    wqkv_d = nc.dram_tensor("wT_qkv", (D, 3 * D), F32, kind="ExternalInput")
    bqkv_d = nc.dram_tensor("b_qkv", (3 * D,), F32, kind="ExternalInput")
    wproj_d = nc.dram_tensor("wT_proj", (D, D), F32, kind="ExternalInput")
    bproj_d = nc.dram_tensor("b_proj", (D,), F32, kind="ExternalInput")
    out_d = nc.dram_tensor("out", (T, D), F32, kind="ExternalOutput")

    with tile.TileContext(nc) as tc:
        _body(nc, tc, xt_d, wqkv_d, bqkv_d, wproj_d, bproj_d, out_d)
    nc.compile()
    return nc


def _body(nc, tc, xt_d, wqkv_d, bqkv_d, wproj_d, bproj_d, out_d):
    from contextlib import ExitStack
    with ExitStack() as ctx:
        consts = ctx.enter_context(tc.tile_pool(name="consts", bufs=1))
        qkt_pool = ctx.enter_context(tc.tile_pool(name="qkt", bufs=1))
        v_pool = ctx.enter_context(tc.tile_pool(name="vst", bufs=1))
        ot_pool = ctx.enter_context(tc.tile_pool(name="ot", bufs=1))
        x_pool = ctx.enter_context(tc.tile_pool(name="x", bufs=1))
        wqk_pool = ctx.enter_context(tc.tile_pool(name="wqk", bufs=1))
        mm_ps = ctx.enter_context(tc.tile_pool(name="mmps", bufs=2, space="PSUM"))

        bias_qk = consts.tile([128, 12], F32)
        nc.sync.dma_start(bias_qk[:], bqkv_d[0:1536].rearrange("(t p) -> p t", p=128))
        bias_v = consts.tile([128, D], F32)
        nc.sync.dma_start(bias_v[:], _bcast_ap(bqkv_d[1536:2304], 128, D))
        bias_p = consts.tile([128, D], F32)
        nc.sync.dma_start(bias_p[:], _bcast_ap(bproj_d[0:D], 128, D))
        ones12 = consts.tile([128, H, 1], F32)
        nc.vector.memset(ones12[:], 1.0)

        qkT = [qkt_pool.tile([128, T], F32R, name=f"qkT{fi}") for fi in range(12)]
        vst = [v_pool.tile([128, H, 65], F32R, name=f"vst{ti}") for ti in range(TT)]
        OT = [ot_pool.tile([128, T], F32R, name=f"OT{k}") for k in range(DT)]

        xt_r = xt_d.rearrange("(dt p) t -> p dt t", p=128)
        wq_r = wqkv_d.rearrange("(dt p) f -> p dt f", p=128)
        wp_r = wproj_d.rearrange("(dt p) f -> p dt f", p=128)
        xTM = x_pool.tile([128, DT, T], F32R, name="xTM")
        xT = [xTM[:, k, :] for k in range(DT)]
        wTqkM = wqk_pool.tile([128, DT, 1536], F32R, name="wTqkM")
        wTqk = [wTqkM[:, k, :] for k in range(DT)]
        for k in range(DT):
            nc.sync.dma_start(
                xTM[:, k, 0:512], xt_r[:, k, 0:512].bitcast(F32R))

        def emit_fi(fi):
            """One qkT feature tile: stream its wT_qkv column slice, matmul, evac."""
            for k in range(DT):
                nc.sync.dma_start(
                    wTqkM[:, k, 128 * fi:128 * (fi + 1)],
                    wq_r[:, k, 128 * fi:128 * (fi + 1)].bitcast(F32R))
            for c in range(NCH):
                pq = mm_ps.tile([128, TQ], F32, tag="mm", name=f"pq{fi}_{c}")
                for k in range(DT):
                    nc.tensor.matmul(
                        pq[:], wTqk[k][:, 128 * fi:128 * (fi + 1)],
                        xT[k][:, TQ * c:TQ * (c + 1)],
                        start=(k == 0), stop=(k == DT - 1))
                nc.vector.tensor_scalar_add(
                    qkT[fi][:, TQ * c:TQ * (c + 1)], pq[:], bias_qk[:, fi:fi + 1])

        def emit_v_half(c2, wTv, ti_range=None):
            """v columns [384*c2, 384*(c2+1)) for token tiles (heads 6c2..6c2+6)."""
            for ti in (ti_range if ti_range is not None else range(TT)):
                pv = mm_ps.tile([128, 384], F32, tag="mm", name=f"pv{ti}_{c2}")
                for k in range(DT):
                    nc.tensor.matmul(
                        pv[:], xT[k][:, 128 * ti:128 * (ti + 1)],
                        wTv[k][:], start=(k == 0), stop=(k == DT - 1))
                nc.vector.tensor_add(
                    vst[ti][:, 6 * c2:6 * (c2 + 1), 0:64],
                    pv[:].rearrange("p (h d) -> p h d", d=64),
                    bias_v[:, 384 * c2:384 * (c2 + 1)].rearrange(
                        "p (h d) -> p h d", d=64))
                nc.vector.tensor_copy(
                    vst[ti][:, 6 * c2:6 * (c2 + 1), 64:65], ones12[:, 0:6, :])

        wshare = ctx.enter_context(tc.tile_pool(name="wshare", bufs=2))

        def load_wv(c2):
            m = wshare.tile([128, DT, 384], F32R, tag="ws", name=f"wTvM{c2}")
            for k in range(DT):
                nc.sync.dma_start(
                    m[:, k, :],
                    wq_r[:, k, 1536 + 384 * c2:1536 + 384 * (c2 + 1)].bitcast(F32R))
            return [m[:, k, :] for k in range(DT)]

        # lead-in: just the first head pair's q/k; everything else becomes
        # lower-priority fill-in jobs inside the attention loop.
        wTv0 = load_wv(0)
        for k in range(DT):
            nc.sync.dma_start(
                xTM[:, k, 512:1024], xt_r[:, k, 512:1024].bitcast(F32R))
        emit_fi(0)
        emit_fi(6)
        emit_v_half(0, wTv0)

        # deferred jobs, spread across attention iterations (due-date ordered)
        wTv1 = load_wv(1)
        wTp = {}

        def load_wp():
            for c2 in range(2):
                m = wshare.tile([128, DT, 384], F32R, tag="ws", name=f"wTpM{c2}")
                for k in range(DT):
                    nc.sync.dma_start(
                        m[:, k, :], wp_r[:, k, 384 * c2:384 * (c2 + 1)].bitcast(F32R))
                    wTp[(c2, k)] = m[:, k, :]

        jobs = {
            0: [lambda: emit_fi(1), lambda: emit_fi(7)],
            1: [lambda: emit_fi(2), lambda: emit_fi(8)],
            2: [lambda: emit_fi(3), lambda: emit_fi(9),
                lambda: emit_v_half(1, wTv1)],
            3: [lambda: emit_fi(4), lambda: emit_fi(10)],
            4: [lambda: emit_fi(5), lambda: emit_fi(11)],
            5: [lambda: load_wp()],
        }

        with ExitStack() as ctx2:
            pt_pool = ctx2.enter_context(tc.tile_pool(name="pt", bufs=2))
            sums_pool = ctx2.enter_context(tc.tile_pool(name="sums", bufs=1))
            rsb_pool = ctx2.enter_context(tc.tile_pool(name="rsb", bufs=2))
            s_ps = ctx2.enter_context(tc.tile_pool(name="sps", bufs=2, space="PSUM"))
            o_ps = ctx2.enter_context(tc.tile_pool(name="ops", bufs=1, space="PSUM"))

            for hp in range(6):
                for c in range(NCH):
                    po = [o_ps.tile([128, TQ], F32, tag=f"o{p}",
                                    name=f"ops{c}_{hp}_{p}") for p in (0, 1)]
                    for g, (g0, g1) in enumerate(SG):
                        gl = g1 - g0
                        sp = [s_ps.tile([128, 512 * gl], F32, tag="s",
                                        name=f"sps{c}_{hp}_{g}_{p}") for p in (0, 1)]
                        for tkt in range(g0, g1):
                            for p in (0, 1):
                                qb = 64 * p
                                nc.tensor.matmul(
                                    sp[p][:, 512 * (tkt - g0):512 * (tkt - g0 + 1)],
                                    qkT[6 + hp][qb:qb + 64, 128 * tkt:128 * (tkt + 1)],
                                    qkT[hp][qb:qb + 64, TQ * c:TQ * (c + 1)],
                                    start=True, stop=True)
                        pt = [pt_pool.tile([128, 512 * gl], F32R, tag=f"pt{p}",
                                           name=f"PT{c}_{hp}_{g}_{p}") for p in (0, 1)]
                        for p in (0, 1):
                            nc.scalar.activation(
                                pt[p][:], sp[p][:],
                                mybir.ActivationFunctionType.Exp,
                                bias=0.0, scale=float(SCALE))
                        for p in (0, 1):
                            h = 2 * hp + p
                            for tk in range(g0, g1):
                                nc.tensor.matmul(
                                    po[p][0:65, :], vst[tk][:, h, :],
                                    pt[p][:, 512 * (tk - g0):512 * (tk - g0 + 1)],
                                    start=(g == 0 and tk == g0),
                                    stop=(g == len(SG) - 1 and tk == g1 - 1),
                                    skip_group_check=True)
                    for p in (0, 1):
                        sst = sums_pool.tile([128, TQ], F32, tag="sums",
                                             name=f"sst{c}_{hp}_{p}")
                        nc.vector.tensor_copy(sst[0:1, :], po[p][64:65, :])
                        oc = sums_pool.tile([64, TQ], F32, tag="oc", bufs=2,
                                            name=f"oc{c}_{hp}_{p}")
                        nc.vector.tensor_copy(oc[:], po[p][0:64, :])
                        nc.vector.reciprocal_approx_fast(sst[0:1, :], sst[0:1, :])
                        rsb = rsb_pool.tile([64, TQ], F32, tag="rsb",
                                            name=f"rsb{c}_{hp}_{p}")
                        nc.gpsimd.partition_broadcast(rsb[:], sst[0:1, :])
                        nc.vector.tensor_mul(
                            OT[hp][64 * p:64 * (p + 1), TQ * c:TQ * (c + 1)],
                            oc[:], rsb[:])
                for job in jobs.get(hp, []):
                    job()

        # ---------------- projection ----------------
        with ExitStack() as ctx3:
            outst = ctx3.enter_context(tc.tile_pool(name="outst", bufs=3))
            for ti in range(TT):
                ob = outst.tile([128, D], F32, tag="ob", name=f"ob{ti}")
                for c2 in range(2):
                    pp = mm_ps.tile([128, 384], F32, tag="mm", name=f"pp{ti}_{c2}")
                    for k in range(DT):
                        nc.tensor.matmul(
                            pp[:], OT[k][:, 128 * ti:128 * (ti + 1)],
                            wTp[(c2, k)][:],
                            start=(k == 0), stop=(k == DT - 1))
                    nc.vector.tensor_add(
                        ob[:, 384 * c2:384 * (c2 + 1)], pp[:],
                        bias_p[:, 384 * c2:384 * (c2 + 1)])
                nc.sync.dma_start(out_d[128 * ti:128 * (ti + 1), :], ob[:])


_CACHE = {}


def _get_runner():
    if "runner" in _CACHE:
        return _CACHE["runner"]
    import jax
    from jax.sharding import Mesh, PartitionSpec
    from jax.experimental.shard_map import shard_map
    from concourse import bass2jax
    from concourse.bass2jax import _bass_exec_p, partition_id_tensor

    nc = build_nc()
    bass2jax.install_neuronx_cc_hook()
    partition_name = nc.partition_id_tensor.name if nc.partition_id_tensor else None
    in_names, out_names, out_avals = [], [], []
    for alloc in nc.m.functions[0].allocations:
        if not isinstance(alloc, mybir.MemoryLocationSet):
            continue
        name = alloc.memorylocations[0].name
        if alloc.kind == "ExternalInput":
            if name != partition_name:
                in_names.append(name)
        elif alloc.kind == "ExternalOutput":
            out_names.append(name)
            out_avals.append(jax.core.ShapedArray(
                tuple(alloc.tensor_shape), mybir.dt.np(alloc.dtype)))
    all_in = list(in_names) + list(out_names)
    if partition_name is not None:
        all_in.append(partition_name)

    def _jbody(*args):
        operands = list(args)
        if partition_name is not None:
            operands.append(partition_id_tensor())
        return tuple(_bass_exec_p.bind(
            *operands, out_avals=tuple(out_avals), in_names=tuple(all_in),
            out_names=tuple(out_names), lowering_input_output_aliases=(),
            sim_require_finite=True, sim_require_nnan=True, nc=nc))

    devices = jax.devices()[:N_CORES]
    mesh = Mesh(np.asarray(devices), ("core",))
    # xT is batch-sharded on the core axis; weights/biases are replicated.
    sharded_in = {"xT"}
    in_specs = tuple(
        PartitionSpec("core") if n in sharded_in else PartitionSpec()
        for n in in_names
    ) + (PartitionSpec("core"),) * len(out_names)
    fn = jax.jit(
        shard_map(_jbody, mesh=mesh, in_specs=in_specs,
                  out_specs=(PartitionSpec("core"),) * len(out_names),
                  check_rep=False),
        keep_unused=True)
    _CACHE["runner"] = (fn, in_names, out_names, out_avals, mesh)
    return _CACHE["runner"]


def _weight_key(*arrs):
    import hashlib
    h = hashlib.sha1()
    for a in arrs:
        h.update(np.ascontiguousarray(a, np.float32).tobytes())
    return h.hexdigest()


def kernel(x, w_qkv, b_qkv, w_proj, b_proj):
    import jax
    fn, in_names, out_names, out_avals, mesh = _get_runner()
    x = np.asarray(x, dtype=np.float32)
    xt = np.ascontiguousarray(np.transpose(x, (0, 2, 1)))        # [B, D, T]
    xt_flat = xt.reshape(N_CORES * D, T)

    wk = _weight_key(w_qkv, b_qkv, w_proj, b_proj)
    if _CACHE.get("wkey") != wk:
        wqt = np.ascontiguousarray(np.asarray(w_qkv, np.float32).T)   # [D, 3D]
        wpt = np.ascontiguousarray(np.asarray(w_proj, np.float32).T)  # [D, D]
        host_w = {
            "wT_qkv": wqt,
            "b_qkv": np.asarray(b_qkv, np.float32),
            "wT_proj": wpt,
            "b_proj": np.asarray(b_proj, np.float32),
        }
        _CACHE["wdev"] = {k: jax.device_put(v) for k, v in host_w.items()}
        _CACHE["wkey"] = wk
    wdev = _CACHE["wdev"]

    args = []
    for n in in_names:
        args.append(xt_flat if n == "xT" else wdev[n])
    for a in out_avals:
        args.append(np.zeros((N_CORES * a.shape[0], *a.shape[1:]), a.dtype))
    outs = fn(*args)
    jax.block_until_ready(outs)
    oi = out_names.index("out")
    return np.asarray(outs[oi]).reshape(N_CORES, T, D).astype(np.float32)


# revision 36
# speedup vs baseline: 1.0266x; 1.0266x over previous
"""Multi-head attention block (nn_Attention) on 8 Trainium2 NeuronCores.

Reference computation (per batch element, all fp32):
    qkv = x @ w_qkv.T + b_qkv               # [T, 3D]
    q, k, v per head (H=12, Hd=64)
    attn = softmax(q @ k.T / sqrt(Hd))
    out  = (attn @ v) @ w_proj.T + b_proj   # [T, D]

Sharding: pure data parallelism over the batch (B=8) — one batch element per
NeuronCore, weights replicated. No collectives.

All matmuls run in float32r (fp32 storage, TF32-like PE mode: full rate for
moving dim >= 256, ~1.6e-4 matmul relative error). x and the weights are
pre-transposed on the host so every operand DMAs in with the contraction
dim on partitions and unit-stride free dims:
    xT  [D, T],  wT_qkv [D, 3D],  wT_proj [D, D]

Per-core pipeline:
  1. qkT [1536, T] = wT_qk.T-contract against xT (features on partitions),
     bias folded into the PSUM evacuation. v_nat [T, 768] = x @ w_v.T,
     staged head-major as [v_h | 1] blocks of 65 columns (the ones column
     makes the attention matmul emit softmax denominators for free).
  2. Per head pair (heads 2i, 2i+1 live at partition bases 0/64 of one
     qkT tile, so their K=64 S.T matmuls occupy distinct PE row groups and
     run concurrently): S.T = kT_h.T @ qT_h per 128-key tile, exp on
     ScalarE (scale=1/8 folded, output rounded to f32r), then
     O'.T [65, tq] = [v_h | 1].T @ P.T accumulated over the 8 key tiles.
     Row 64 of O'.T is the softmax denominator. The evacuation multiplies
     rows 0:63 by the broadcast reciprocal (GPSIMD partition_broadcast)
     into OT [D, T].
  3. out = OT.T-contract against wT_proj + b_proj, written token-major.

QKV matmuls for head pair i+1 are emitted between attention stages of pair
i so the PE stays busy while ScalarE grinds the exps (ScalarE is the
attention-phase bottleneck at ~1 elem/lane/cycle).
"""
import os
import numpy as np

os.environ.setdefault("JAX_COMPILATION_CACHE_DIR", "/tmp/jax_neff_cache")

import concourse.bass as bass
import concourse.bacc as bacc
import concourse.tile as tile
from concourse import mybir

F32 = mybir.dt.float32
F32R = mybir.dt.float32r

B, T, D = 8, 1024, 768
H, HD = 12, 64
SCALE = HD ** -0.5
N_CORES = 8
TT = T // 128       # 8 token tiles
DT = D // 128       # 6 contraction tiles
TQ = 512            # query chunk (moving dim)
NCH = T // TQ       # 2 query chunks
SG = [(0, 2), (2, 4), (4, 6), (6, 8)]  # key-tile groups (2 PSUM banks each)


def _bcast_ap(ap_1d, parts, n):
    return bass.AP(tensor=ap_1d.tensor, offset=ap_1d.offset,
                   ap=[[0, parts], [1, n]])


def build_nc():
    nc = bacc.Bacc(trn_type="TRN2", debug=False, num_devices=N_CORES)
    xt_d = nc.dram_tensor("xT", (D, T), F32, kind="ExternalInput")
    wqkv_d = nc.dram_tensor("wT_qkv", (D, 3 * D), F32, kind="ExternalInput")
    bqkv_d = nc.dram_tensor("b_qkv", (3 * D,), F32, kind="ExternalInput")
    wproj_d = nc.dram_tensor("wT_proj", (D, D), F32, kind="ExternalInput")
    bproj_d = nc.dram_tensor("b_proj", (D,), F32, kind="ExternalInput")
    out_d = nc.dram_tensor("out", (T, D), F32, kind="ExternalOutput")

    with tile.TileContext(nc) as tc:
        _body(nc, tc, xt_d, wqkv_d, bqkv_d, wproj_d, bproj_d, out_d)
    nc.compile()
    return nc


def _body(nc, tc, xt_d, wqkv_d, bqkv_d, wproj_d, bproj_d, out_d):
    from contextlib import ExitStack
    with ExitStack() as ctx:
        consts = ctx.enter_context(tc.tile_pool(name="consts", bufs=1))
        qkt_pool = ctx.enter_context(tc.tile_pool(name="qkt", bufs=1))
        v_pool = ctx.enter_context(tc.tile_pool(name="vst", bufs=1))
        ot_pool = ctx.enter_context(tc.tile_pool(name="ot", bufs=1))
        x_pool = ctx.enter_context(tc.tile_pool(name="x", bufs=1))
        wqk_pool = ctx.enter_context(tc.tile_pool(name="wqk", bufs=1))
        mm_ps = ctx.enter_context(tc.tile_pool(name="mmps", bufs=2, space="PSUM"))

        bias_qk = consts.tile([128, 12], F32)
        nc.sync.dma_start(bias_qk[:], bqkv_d[0:1536].rearrange("(t p) -> p t", p=128))
        bias_v = consts.tile([128, D], F32)
        bias_p = consts.tile([128, D], F32)
        ones12 = consts.tile([128, H, 1], F32)
        nc.vector.memset(ones12[:], 1.0)

        qkT = [qkt_pool.tile([128, T], F32R, name=f"qkT{fi}") for fi in range(12)]
        vst = [v_pool.tile([128, H, 65], F32R, name=f"vst{ti}") for ti in range(TT)]
        OT = [ot_pool.tile([128, T], F32R, name=f"OT{k}") for k in range(DT)]

        xt_r = xt_d.rearrange("(dt p) t -> p dt t", p=128)
        wq_r = wqkv_d.rearrange("(dt p) f -> p dt f", p=128)
        wp_r = wproj_d.rearrange("(dt p) f -> p dt f", p=128)
        xTM = x_pool.tile([128, DT, T], F32R, name="xTM")
        xT = [xTM[:, k, :] for k in range(DT)]
        wTqkM = wqk_pool.tile([128, DT, 1536], F32R, name="wTqkM")
        wTqk = [wTqkM[:, k, :] for k in range(DT)]
        for k in range(DT):
            nc.scalar.dma_start(
                xTM[:, k, 0:512], xt_r[:, k, 0:512].bitcast(F32R))
        for k in range(DT):
            nc.scalar.dma_start(
                xTM[:, k, 512:1024], xt_r[:, k, 512:1024].bitcast(F32R))

        def emit_fi(fi):
            """One qkT feature tile: stream its wT_qkv column slice, matmul, evac."""
            for k in range(DT):
                nc.sync.dma_start(
                    wTqkM[:, k, 128 * fi:128 * (fi + 1)],
                    wq_r[:, k, 128 * fi:128 * (fi + 1)].bitcast(F32R))
            for c in range(NCH):
                pq = mm_ps.tile([128, TQ], F32, tag="mm", name=f"pq{fi}_{c}")
                for k in range(DT):
                    nc.tensor.matmul(
                        pq[:], wTqk[k][:, 128 * fi:128 * (fi + 1)],
                        xT[k][:, TQ * c:TQ * (c + 1)],
                        start=(k == 0), stop=(k == DT - 1))
                nc.vector.tensor_scalar_add(
                    qkT[fi][:, TQ * c:TQ * (c + 1)], pq[:], bias_qk[:, fi:fi + 1])

        def emit_v_half(c2, wTv, ti_range=None):
            """v columns [384*c2, 384*(c2+1)) for token tiles (heads 6c2..6c2+6)."""
            for ti in (ti_range if ti_range is not None else range(TT)):
                pv = mm_ps.tile([128, 384], F32, tag="mm", name=f"pv{ti}_{c2}")
                for k in range(DT):
                    nc.tensor.matmul(
                        pv[:], xT[k][:, 128 * ti:128 * (ti + 1)],
                        wTv[k][:], start=(k == 0), stop=(k == DT - 1))
                nc.vector.tensor_add(
                    vst[ti][:, 6 * c2:6 * (c2 + 1), 0:64],
                    pv[:].rearrange("p (h d) -> p h d", d=64),
                    bias_v[:, 384 * c2:384 * (c2 + 1)].rearrange(
                        "p (h d) -> p h d", d=64))
                nc.vector.tensor_copy(
                    vst[ti][:, 6 * c2:6 * (c2 + 1), 64:65], ones12[:, 0:6, :])

        wshare = ctx.enter_context(tc.tile_pool(name="wshare", bufs=2))

        def load_wv(c2):
            m = wshare.tile([128, DT, 384], F32R, tag="ws", name=f"wTvM{c2}")
            for k in range(DT):
                nc.sync.dma_start(
                    m[:, k, :],
                    wq_r[:, k, 1536 + 384 * c2:1536 + 384 * (c2 + 1)].bitcast(F32R))
            return [m[:, k, :] for k in range(DT)]

        # attention pools (opened early so the first pair's S/exp can precede
        # v0; closed manually before the projection scope needs the space)
        attn_ctx = ExitStack()
        pt_pool = attn_ctx.enter_context(tc.tile_pool(name="pt", bufs=2))
        sums_pool = attn_ctx.enter_context(tc.tile_pool(name="sums", bufs=1))
        rsb_pool = attn_ctx.enter_context(tc.tile_pool(name="rsb", bufs=2))
        s_ps = attn_ctx.enter_context(tc.tile_pool(name="sps", bufs=2, space="PSUM"))
        o_ps = attn_ctx.enter_context(tc.tile_pool(name="ops", bufs=1, space="PSUM"))

        def emit_sg(hp, c, g):
            """S.T matmuls + exp for one key-tile group of a head pair."""
            g0, g1 = SG[g]
            gl = g1 - g0
            sp = [s_ps.tile([128, 512 * gl], F32, tag="s",
                            name=f"sps{c}_{hp}_{g}_{p}") for p in (0, 1)]
            for tkt in range(g0, g1):
                for p in (0, 1):
                    qb = 64 * p
                    nc.tensor.matmul(
                        sp[p][:, 512 * (tkt - g0):512 * (tkt - g0 + 1)],
                        qkT[6 + hp][qb:qb + 64, 128 * tkt:128 * (tkt + 1)],
                        qkT[hp][qb:qb + 64, TQ * c:TQ * (c + 1)],
                        start=True, stop=True)
            pt = [pt_pool.tile([128, 512 * gl], F32R, tag=f"pt{p}",
                               name=f"PT{c}_{hp}_{g}_{p}") for p in (0, 1)]
            for p in (0, 1):
                nc.scalar.activation(
                    pt[p][:], sp[p][:], mybir.ActivationFunctionType.Exp,
                    bias=0.0, scale=float(SCALE))
            return pt

        def emit_og(hp, c, po, g, pt):
            g0, g1 = SG[g]
            for p in (0, 1):
                h = 2 * hp + p
                for tk in range(g0, g1):
                    nc.tensor.matmul(
                        po[p][0:65, :], vst[tk][:, h, :],
                        pt[p][:, 512 * (tk - g0):512 * (tk - g0 + 1)],
                        start=(g == 0 and tk == g0),
                        stop=(g == len(SG) - 1 and tk == g1 - 1),
                        skip_group_check=True)

        def emit_norm(hp, c, po):
            for p in (0, 1):
                sst = sums_pool.tile([128, TQ], F32, tag="sums",
                                     name=f"sst{c}_{hp}_{p}")
                nc.vector.tensor_copy(sst[0:1, :], po[p][64:65, :])
                oc = sums_pool.tile([64, TQ], F32, tag="oc", bufs=2,
                                    name=f"oc{c}_{hp}_{p}")
                nc.vector.tensor_copy(oc[:], po[p][0:64, :])
                nc.vector.reciprocal_approx_fast(sst[0:1, :], sst[0:1, :])
                rsb = rsb_pool.tile([64, TQ], F32, tag="rsb",
                                    name=f"rsb{c}_{hp}_{p}")
                nc.gpsimd.partition_broadcast(rsb[:], sst[0:1, :])
                nc.vector.tensor_mul(
                    OT[hp][64 * p:64 * (p + 1), TQ * c:TQ * (c + 1)],
                    oc[:], rsb[:])

        def emit_attn(hp, c, po, pre_pt=()):
            for g in range(len(SG)):
                pt = pre_pt[g] if g < len(pre_pt) else emit_sg(hp, c, g)
                emit_og(hp, c, po, g, pt)
            emit_norm(hp, c, po)

        # lead-in: first head pair's q/k, then its first S/exp groups so
        # ScalarE ramps while the PE grinds the v projection.
        emit_fi(0)
        emit_fi(6)
        po_pre = [o_ps.tile([128, TQ], F32, tag=f"o{p}", name=f"ops0_0_{p}")
                  for p in (0, 1)]
        pre_pt = [emit_sg(0, 0, 0), emit_sg(0, 0, 1)]
        nc.sync.dma_start(bias_v[:], _bcast_ap(bqkv_d[1536:2304], 128, D))
        nc.sync.dma_start(bias_p[:], _bcast_ap(bproj_d[0:D], 128, D))
        wTv0 = load_wv(0)
        # interleave the v projection with head pair 0 / chunk 0 attention so
        # ScalarE's exp stream starts immediately
        emit_v_half(0, wTv0, ti_range=range(0, 2))
        emit_og(0, 0, po_pre, 0, pre_pt[0])
        emit_v_half(0, wTv0, ti_range=range(2, 4))
        pre_pt.append(emit_sg(0, 0, 2))
        emit_og(0, 0, po_pre, 1, pre_pt[1])
        emit_v_half(0, wTv0, ti_range=range(4, 6))
        pre_pt.append(emit_sg(0, 0, 3))
        emit_og(0, 0, po_pre, 2, pre_pt[2])
        emit_v_half(0, wTv0, ti_range=range(6, 8))
        emit_og(0, 0, po_pre, 3, pre_pt[3])
        emit_norm(0, 0, po_pre)

        # deferred jobs, spread across attention iterations (due-date ordered)
        wTv1 = load_wv(1)
        wTp = {}

        def load_wp():
            for c2 in range(2):
                m = wshare.tile([128, DT, 384], F32R, tag="ws", name=f"wTpM{c2}")
                for k in range(DT):
                    nc.sync.dma_start(
                        m[:, k, :], wp_r[:, k, 384 * c2:384 * (c2 + 1)].bitcast(F32R))
                    wTp[(c2, k)] = m[:, k, :]

        jobs = {
            0: [lambda: emit_fi(1), lambda: emit_fi(7)],
            1: [lambda: emit_fi(2), lambda: emit_fi(8)],
            2: [lambda: emit_fi(3), lambda: emit_fi(9),
                lambda: emit_v_half(1, wTv1)],
            3: [lambda: emit_fi(4), lambda: emit_fi(10)],
            4: [lambda: emit_fi(5), lambda: emit_fi(11)],
            5: [lambda: load_wp()],
        }

        for hp in range(6):
            for c in range(NCH):
                if hp == 0 and c == 0:
                    continue
                po = [o_ps.tile([128, TQ], F32, tag=f"o{p}",
                                name=f"ops{c}_{hp}_{p}") for p in (0, 1)]
                emit_attn(hp, c, po)
            for job in jobs.get(hp, []):
                job()
        attn_ctx.close()

        # ---------------- projection ----------------
        with ExitStack() as ctx3:
            outst = ctx3.enter_context(tc.tile_pool(name="outst", bufs=3))
            for ti in range(TT):
                ob = outst.tile([128, D], F32, tag="ob", name=f"ob{ti}")
                for c2 in range(2):
                    pp = mm_ps.tile([128, 384], F32, tag="mm", name=f"pp{ti}_{c2}")
                    for k in range(DT):
                        nc.tensor.matmul(
                            pp[:], OT[k][:, 128 * ti:128 * (ti + 1)],
                            wTp[(c2, k)][:],
                            start=(k == 0), stop=(k == DT - 1))
                    nc.vector.tensor_add(
                        ob[:, 384 * c2:384 * (c2 + 1)], pp[:],
                        bias_p[:, 384 * c2:384 * (c2 + 1)])
                nc.sync.dma_start(out_d[128 * ti:128 * (ti + 1), :], ob[:])


_CACHE = {}


def _get_runner():
    if "runner" in _CACHE:
        return _CACHE["runner"]
    import jax
    from jax.sharding import Mesh, PartitionSpec
    from jax.experimental.shard_map import shard_map
    from concourse import bass2jax
    from concourse.bass2jax import _bass_exec_p, partition_id_tensor

    nc = build_nc()
    bass2jax.install_neuronx_cc_hook()
    partition_name = nc.partition_id_tensor.name if nc.partition_id_tensor else None
    in_names, out_names, out_avals = [], [], []
    for alloc in nc.m.functions[0].allocations:
        if not isinstance(alloc, mybir.MemoryLocationSet):
            continue
        name = alloc.memorylocations[0].name
        if alloc.kind == "ExternalInput":
            if name != partition_name:
                in_names.append(name)
        elif alloc.kind == "ExternalOutput":
            out_names.append(name)
            out_avals.append(jax.core.ShapedArray(
                tuple(alloc.tensor_shape), mybir.dt.np(alloc.dtype)))
    all_in = list(in_names) + list(out_names)
    if partition_name is not None:
        all_in.append(partition_name)

    def _jbody(*args):
        operands = list(args)
        if partition_name is not None:
            operands.append(partition_id_tensor())
        return tuple(_bass_exec_p.bind(
            *operands, out_avals=tuple(out_avals), in_names=tuple(all_in),
            out_names=tuple(out_names), lowering_input_output_aliases=(),
            sim_require_finite=True, sim_require_nnan=True, nc=nc))

    devices = jax.devices()[:N_CORES]
    mesh = Mesh(np.asarray(devices), ("core",))
    # xT is batch-sharded on the core axis; weights/biases are replicated.
    sharded_in = {"xT"}
    in_specs = tuple(
        PartitionSpec("core") if n in sharded_in else PartitionSpec()
        for n in in_names
    ) + (PartitionSpec("core"),) * len(out_names)
    fn = jax.jit(
        shard_map(_jbody, mesh=mesh, in_specs=in_specs,
                  out_specs=(PartitionSpec("core"),) * len(out_names),
                  check_rep=False),
        keep_unused=True)
    _CACHE["runner"] = (fn, in_names, out_names, out_avals, mesh)
    return _CACHE["runner"]


def _weight_key(*arrs):
    import hashlib
    h = hashlib.sha1()
    for a in arrs:
        h.update(np.ascontiguousarray(a, np.float32).tobytes())
    return h.hexdigest()


def kernel(x, w_qkv, b_qkv, w_proj, b_proj):
    import jax
    fn, in_names, out_names, out_avals, mesh = _get_runner()
    x = np.asarray(x, dtype=np.float32)
    xt = np.ascontiguousarray(np.transpose(x, (0, 2, 1)))        # [B, D, T]
    xt_flat = xt.reshape(N_CORES * D, T)

    wk = _weight_key(w_qkv, b_qkv, w_proj, b_proj)
    if _CACHE.get("wkey") != wk:
        wqt = np.ascontiguousarray(np.asarray(w_qkv, np.float32).T)   # [D, 3D]
        wpt = np.ascontiguousarray(np.asarray(w_proj, np.float32).T)  # [D, D]
        host_w = {
            "wT_qkv": wqt,
            "b_qkv": np.asarray(b_qkv, np.float32),
            "wT_proj": wpt,
            "b_proj": np.asarray(b_proj, np.float32),
        }
        _CACHE["wdev"] = {k: jax.device_put(v) for k, v in host_w.items()}
        _CACHE["wkey"] = wk
    wdev = _CACHE["wdev"]

    args = []
    for n in in_names:
        args.append(xt_flat if n == "xT" else wdev[n])
    for a in out_avals:
        args.append(np.zeros((N_CORES * a.shape[0], *a.shape[1:]), a.dtype))
    outs = fn(*args)
    jax.block_until_ready(outs)
    oi = out_names.index("out")
    return np.asarray(outs[oi]).reshape(N_CORES, T, D).astype(np.float32)
